# revision 11
# baseline (speedup 1.0000x reference)
"""Trainium2 Bass kernel for PerObjectEpisodicMemory scatter-store.

Semantics (per object m, all indices unique):
    cnt = slot_count[m]
    deltas[k] = ||R_cam @ R_s[m,k]^T - I||_F + ||t_cam - t_s[m,k]||
    slot = cnt < K ? cnt : argmin_k deltas
    memory[m, slot] = features[m]
    capture_poses[m, slot] = camera_pose
    slot_filled[m, slot] = True
    slot_count[m] += (cnt < K)

Sharding: object axis split contiguously across 8 NeuronCores, camera_pose
replicated. Because every object receives at most one slot write, the scatter
is implemented as a predicated overwrite (copy_predicated) on tiles streamed
through SBUF -- no indirect DMA. General (non-arange) unique object_indices
are handled by a host-side pre-scatter of features plus a per-object update
mask fed to the device.
"""

import numpy as np

N_CORES = 8
MAX_OBJECTS = 65536
K = 8
D = 256
P = 128
M_CORE = MAX_OBJECTS // N_CORES  # 8192
T = M_CORE // P  # 64

_CACHE = {}


def _build_bass():
    import concourse.mybir as mybir
    from concourse import bacc
    from concourse.masks import make_identity
    from concourse.tile import TileContext

    f32 = mybir.dt.float32
    i32 = mybir.dt.int32
    u8 = mybir.dt.uint8
    OP = mybir.AluOpType
    AX = mybir.AxisListType
    AF = mybir.ActivationFunctionType

    nc = bacc.Bacc("TRN2")

    mem_in = nc.dram_tensor("mem_in", [M_CORE, K * D], f32, kind="ExternalInput")
    poses_in = nc.dram_tensor("poses_in", [M_CORE, K * 16], f32, kind="ExternalInput")
    filled_in = nc.dram_tensor("filled_in", [M_CORE, K], u8, kind="ExternalInput")
    cnt_in = nc.dram_tensor("cnt_in", [M_CORE], i32, kind="ExternalInput")
    feat_in = nc.dram_tensor("feat_in", [M_CORE, D], f32, kind="ExternalInput")
    cam_in = nc.dram_tensor("cam_in", [16], f32, kind="ExternalInput")
    upd_in = nc.dram_tensor("upd_in", [M_CORE], i32, kind="ExternalInput")

    mem_out = nc.dram_tensor("mem_out", [M_CORE, K * D], f32, kind="ExternalOutput")
    poses_out = nc.dram_tensor("poses_out", [M_CORE, K * 16], f32, kind="ExternalOutput")
    filled_out = nc.dram_tensor("filled_out", [M_CORE, K], u8, kind="ExternalOutput")
    cnt_out = nc.dram_tensor("cnt_out", [M_CORE], i32, kind="ExternalOutput")

    with TileContext(nc) as tc:
        with (
            tc.tile_pool(name="small", bufs=1) as sp,
            tc.tile_pool(name="work", bufs=3) as wp,
            tc.tile_pool(name="mem", bufs=4) as mp,
            tc.tile_pool(name="psum", bufs=2, space="PSUM") as pp,
        ):
            # ---- constants ----
            # Instructions reading SBUF operands written by different DMA
            # queues can exceed the per-instruction sync-wait budget (the
            # TensorScalarPtr encoding fits a single wait). Route every
            # small DMA/gpsimd-produced operand through a DVE copy so DVE
            # consumers only ever wait on at most one DMA semaphore.
            cam0 = sp.tile([P, 16], f32)
            nc.sync.dma_start(out=cam0[:], in_=cam_in[None, :].to_broadcast((P, 16)))
            cam = sp.tile([P, 16], f32)
            nc.vector.tensor_copy(cam[:], cam0[:])
            kidx_i = sp.tile([P, K], i32)
            nc.gpsimd.iota(kidx_i[:], pattern=[[1, K]], channel_multiplier=0)
            kidx = sp.tile([P, K], f32)
            nc.vector.tensor_copy(kidx[:], kidx_i[:])
            ident0 = sp.tile([P, P], f32)
            make_identity(nc, ident0[:])
            ident = sp.tile([P, P], f32)
            nc.vector.tensor_copy(ident[:], ident0[:])

            # ---- pose slab: partition = object mod 128, cols = (t, k, e) ----
            ptile = sp.tile([P, T * K * 16], f32)

            def p4():
                return ptile[:].rearrange("p (t k e) -> p t k e", k=K, e=16)

            nc.sync.dma_start(
                out=p4(),
                in_=poses_in.rearrange("(t p) (k e) -> p t k e", p=P, e=16),
            )
            # Sacrificial DVE read so the pose-DMA semaphore is observed once;
            # later DVE consumers then only need the (single-slot) DVE wait.
            ptouch = sp.tile([P, 1], f32)
            nc.vector.tensor_copy(ptouch[:], ptile[:, 0:1])

            # Preload the full features slab once (8MB) so the memory loop's
            # copy_predicated has a single fresh (DMA) dependency per tile.
            feat_all = sp.tile([P, T * D], f32)
            nc.sync.dma_start(
                out=feat_all[:].rearrange("p (t d) -> p t d", d=D),
                in_=feat_in.rearrange("(t p) d -> p t d", p=P),
            )
            ftouch = sp.tile([P, 1], f32)
            nc.vector.tensor_copy(ftouch[:], feat_all[:, 0:1])

            def pose_el(e):
                # (P, T, K) view of pose element e (0..15)
                return p4()[:, :, :, e]

            # ---- rot_delta^2 = sum_{i,l} (sum_j R[i,j] * Rs[l,j] - I_il)^2 ----
            rotsq = sp.tile([P, T * K], f32)
            first = True
            for i in range(3):
                for l in range(3):
                    acc = wp.tile([P, T * K], f32, tag="acc")
                    acc3 = acc[:].rearrange("p (t k) -> p t k", k=K)
                    if i == l:
                        nc.vector.tensor_scalar(
                            out=acc3,
                            in0=pose_el(l * 4 + 0),
                            scalar1=cam[:, i * 4 + 0 : i * 4 + 1],
                            scalar2=-1.0,
                            op0=OP.mult,
                            op1=OP.add,
                        )
                    else:
                        nc.vector.tensor_scalar(
                            out=acc3,
                            in0=pose_el(l * 4 + 0),
                            scalar1=cam[:, i * 4 + 0 : i * 4 + 1],
                            scalar2=None,
                            op0=OP.mult,
                        )
                    for j in (1, 2):
                        nc.vector.scalar_tensor_tensor(
                            out=acc3,
                            in0=pose_el(l * 4 + j),
                            scalar=cam[:, i * 4 + j : i * 4 + j + 1],
                            in1=acc3,
                            op0=OP.mult,
                            op1=OP.add,
                        )
                    if first:
                        nc.scalar.activation(out=rotsq[:], in_=acc[:], func=AF.Square)
                        first = False
                    else:
                        sq = wp.tile([P, T * K], f32, tag="sq")
                        nc.scalar.activation(out=sq[:], in_=acc[:], func=AF.Square)
                        nc.vector.tensor_tensor(
                            out=rotsq[:], in0=rotsq[:], in1=sq[:], op=OP.add
                        )
            nc.scalar.activation(out=rotsq[:], in_=rotsq[:], func=AF.Sqrt)

            # ---- trans_delta^2 = sum_i (t_s[i] - t_cam[i])^2 ----
            transq = sp.tile([P, T * K], f32)
            first = True
            for i in range(3):
                tmp = wp.tile([P, T * K], f32, tag="tmp")
                tmp3 = tmp[:].rearrange("p (t k) -> p t k", k=K)
                nc.vector.tensor_scalar(
                    out=tmp3,
                    in0=pose_el(i * 4 + 3),
                    scalar1=cam[:, i * 4 + 3 : i * 4 + 4],
                    scalar2=None,
                    op0=OP.subtract,
                )
                if first:
                    nc.scalar.activation(out=transq[:], in_=tmp[:], func=AF.Square)
                    first = False
                else:
                    sq2 = wp.tile([P, T * K], f32, tag="sq2")
                    nc.scalar.activation(out=sq2[:], in_=tmp[:], func=AF.Square)
                    nc.vector.tensor_tensor(
                        out=transq[:], in0=transq[:], in1=sq2[:], op=OP.add
                    )
            nc.scalar.activation(out=transq[:], in_=transq[:], func=AF.Sqrt)

            # delta = rot + trans (reuse rotsq)
            delta = rotsq
            nc.vector.tensor_tensor(
                out=delta[:], in0=delta[:], in1=transq[:], op=OP.add
            )
            delta3 = delta[:].rearrange("p (t k) -> p t k", k=K)

            # ---- argmin_k (first occurrence) ----
            dmin = sp.tile([P, T], f32)
            nc.vector.tensor_reduce(out=dmin[:], in_=delta3, op=OP.min, axis=AX.X)
            esel = sp.tile([P, T * K], f32)
            esel3 = esel[:].rearrange("p (t k) -> p t k", k=K)
            nc.vector.tensor_tensor(
                out=esel3,
                in0=delta3,
                in1=dmin[:, :, None].broadcast_to((P, T, K)),
                op=OP.is_equal,
            )
            # eq=1 -> 0 ; eq=0 -> 1e9
            nc.vector.tensor_scalar(
                out=esel[:],
                in0=esel[:],
                scalar1=-1e9,
                scalar2=1e9,
                op0=OP.mult,
                op1=OP.add,
            )
            nc.vector.tensor_tensor(
                out=esel3,
                in0=esel3,
                in1=kidx[:, None, :].broadcast_to((P, T, K)),
                op=OP.add,
            )
            evict = sp.tile([P, T], f32)
            nc.vector.tensor_reduce(out=evict[:], in_=esel3, op=OP.min, axis=AX.X)

            # ---- slot_count / update mask: load (T,P), cast, PE-transpose ----
            def load_T(dram_vec, name_tag):
                raw = sp.tile([T, P], i32, tag=name_tag + "_raw")
                nc.sync.dma_start(
                    out=raw[:], in_=dram_vec.rearrange("(t p) -> t p", p=P)
                )
                flt = sp.tile([T, P], f32, tag=name_tag + "_f")
                nc.vector.tensor_copy(flt[:], raw[:])
                ps = pp.tile([P, T], f32, tag=name_tag + "_ps")
                nc.tensor.transpose(out=ps[:], in_=flt[:], identity=ident[:T, :T])
                out = sp.tile([P, T], f32, tag=name_tag + "_T")
                nc.vector.tensor_copy(out[:], ps[:])
                return out

            cntT = load_T(cnt_in, "cnt")
            updT = load_T(upd_in, "upd")

            fillf = sp.tile([P, T], f32)
            nc.vector.tensor_scalar(
                out=fillf[:], in0=cntT[:], scalar1=float(K), scalar2=None, op0=OP.is_lt
            )
            nc.vector.tensor_tensor(out=fillf[:], in0=fillf[:], in1=updT[:], op=OP.mult)
            slot = sp.tile([P, T], f32)
            nc.vector.tensor_tensor(out=slot[:], in0=cntT[:], in1=evict[:], op=OP.subtract)
            nc.vector.tensor_tensor(out=slot[:], in0=slot[:], in1=fillf[:], op=OP.mult)
            nc.vector.tensor_tensor(out=slot[:], in0=slot[:], in1=evict[:], op=OP.add)

            # count_new = cnt + fill ; transpose back and store
            cntn = sp.tile([P, T], f32)
            nc.vector.tensor_tensor(out=cntn[:], in0=cntT[:], in1=fillf[:], op=OP.add)
            cntn_ps = pp.tile([T, P], f32)
            nc.tensor.transpose(out=cntn_ps[:], in_=cntn[:], identity=ident[:])
            cnt_o = sp.tile([T, P], i32)
            nc.vector.tensor_copy(cnt_o[:], cntn_ps[:])
            nc.sync.dma_start(
                out=cnt_out.rearrange("(t p) -> t p", p=P), in_=cnt_o[:]
            )

            # ---- write mask per (object, k), gated by update mask ----
            mask = sp.tile([P, T * K], f32)
            mask3 = mask[:].rearrange("p (t k) -> p t k", k=K)
            nc.vector.tensor_tensor(
                out=mask3,
                in0=slot[:, :, None].broadcast_to((P, T, K)),
                in1=kidx[:, None, :].broadcast_to((P, T, K)),
                op=OP.is_equal,
            )
            nc.vector.tensor_tensor(
                out=mask3,
                in0=mask3,
                in1=updT[:, :, None].broadcast_to((P, T, K)),
                op=OP.mult,
            )

            # uint8 mask: CopyPredicated requires an integer mask dtype
            mask_u8 = sp.tile([P, T * K], u8)
            nc.vector.tensor_copy(mask_u8[:], mask[:])
            masku3 = mask_u8[:].rearrange("p (t k) -> p t k", k=K)

            # ---- slot_filled |= mask ----
            fil = sp.tile([P, T * K], u8)
            fil3 = fil[:].rearrange("p (t k) -> p t k", k=K)
            nc.sync.dma_start(
                out=fil3, in_=filled_in.rearrange("(t p) k -> p t k", p=P)
            )
            fltouch = sp.tile([P, 1], u8)
            nc.vector.tensor_copy(fltouch[:], fil[:, 0:1])
            nc.vector.tensor_tensor(out=fil[:], in0=fil[:], in1=mask_u8[:], op=OP.max)
            nc.sync.dma_start(
                out=filled_out.rearrange("(t p) k -> p t k", p=P), in_=fil3
            )

            # ---- capture_poses: predicated overwrite in place, then store ----
            for k in range(K):
                nc.vector.copy_predicated(
                    out=p4()[:, :, k, :],
                    mask=masku3[:, :, k, None].broadcast_to((P, T, 16)),
                    data=cam[:, None, :].broadcast_to((P, T, 16)),
                )
            nc.sync.dma_start(
                out=poses_out.rearrange("(t p) (k e) -> p t k e", p=P, e=16),
                in_=p4(),
            )

            # ---- memory: stream 1MB tiles, predicated overwrite ----
            for t in range(T):
                mt = mp.tile([P, K * D], f32, tag="mt")
                nc.sync.dma_start(out=mt[:], in_=mem_in[t * P : (t + 1) * P, :])
                mt3 = mt[:].rearrange("p (k d) -> p k d", k=K)
                nc.vector.copy_predicated(
                    out=mt3,
                    mask=masku3[:, t, :, None].broadcast_to((P, K, D)),
                    data=feat_all[:, t * D : (t + 1) * D][:, None, :].broadcast_to(
                        (P, K, D)
                    ),
                )
                nc.sync.dma_start(out=mem_out[t * P : (t + 1) * P, :], in_=mt[:])

    nc.compile()
    return nc


def _get_nc():
    if "nc" not in _CACHE:
        _CACHE["nc"] = _build_bass()
    return _CACHE["nc"]


def run_full(inputs, trace=False):
    """Run on 8 cores. inputs: dict as from setup_inputs() (np or jax arrays).

    Returns (outputs_tuple, BassKernelResults)."""
    from concourse.bass_utils import run_bass_kernel_spmd

    memory = np.ascontiguousarray(np.asarray(inputs["memory"], dtype=np.float32)).reshape(
        MAX_OBJECTS, K * D
    )
    poses = np.ascontiguousarray(
        np.asarray(inputs["capture_poses"], dtype=np.float32)
    ).reshape(MAX_OBJECTS, K * 16)
    filled = np.asarray(inputs["slot_filled"]).astype(np.uint8).reshape(MAX_OBJECTS, K)
    cnt = np.ascontiguousarray(np.asarray(inputs["slot_count"], dtype=np.int32))
    obj = np.asarray(inputs["object_indices"]).astype(np.int64)
    feats = np.asarray(inputs["features"], dtype=np.float32)
    cam = np.asarray(inputs["camera_pose"], dtype=np.float32).reshape(16)

    # Route each feature row to its object row; mark updated objects.
    feat_full = np.zeros((MAX_OBJECTS, D), np.float32)
    upd = np.zeros((MAX_OBJECTS,), np.int32)
    feat_full[obj] = feats
    upd[obj] = 1

    in_maps = []
    for c in range(N_CORES):
        s = slice(c * M_CORE, (c + 1) * M_CORE)
        in_maps.append(
            {
                "mem_in": np.ascontiguousarray(memory[s]),
                "poses_in": np.ascontiguousarray(poses[s]),
                "filled_in": np.ascontiguousarray(filled[s]),
                "cnt_in": np.ascontiguousarray(cnt[s]),
                "feat_in": np.ascontiguousarray(feat_full[s]),
                "cam_in": cam,
                "upd_in": np.ascontiguousarray(upd[s]),
            }
        )

    res = run_bass_kernel_spmd(
        _get_nc(), in_maps, core_ids=list(range(N_CORES)), trace=trace
    )
    r = res.results
    mem_new = np.concatenate([x["mem_out"] for x in r]).reshape(MAX_OBJECTS, K, D)
    poses_new = np.concatenate([x["poses_out"] for x in r]).reshape(
        MAX_OBJECTS, K, 4, 4
    )
    filled_new = np.concatenate([x["filled_out"] for x in r]).astype(bool)
    cnt_new = np.concatenate([x["cnt_out"] for x in r]).astype(np.int32)
    return (mem_new, poses_new, filled_new, cnt_new), res


def kernel(**inputs):
    outs, _ = run_full(inputs, trace=False)
    return outs
